# revision 1
# baseline (speedup 1.0000x reference)
"""Biaffine attention kernel for Trainium2, data-parallel over 8 NeuronCores.

Math (per batch b):
    xp = Wf @ x[b] + bf          (128, L)
    yp = Wa @ y[b] + ba          (128, L)
    scores = xp @ yp.T           (128, 128)   contraction over L
    attn = softmax(scores, -1) / sqrt(L)
    out[b] = attn @ (xp + yp)    (128, L)

Distribution: batch dim (32) sharded 4-per-core across 8 cores; weights
replicated. No collectives.

Per-core dataflow:
  - x/y streamed HBM->SBUF in 2 MiB tiles (fp32).
  - projections on TensorE as float32r (FP22 single-pass) matmuls, N=512.
  - PSUM evacuated by ScalarE with fused per-partition bias add, cast to
    fp16 activations (xp16/yp16) kept resident in SBUF for the whole batch.
  - xp16/yp16 transposed 128x128 via TensorE transpose-mode (fp16), PSUM
    evacuated by VectorE into xpT/ypT; scores accumulate over 64 chunks
    into one PSUM bank (fp16 matmuls).
  - softmax rowwise (free dim): DVE max-reduce, ACT exp with fused
    -max bias and sum accumulation, DVE reciprocal; 1/sqrt(L) folded in.
  - out = attnT.T @ xp16 + attnT.T @ yp16 accumulated in PSUM, evacuated
    and DMA'd back as fp32.
"""

import numpy as np

P = 128
L = 8192
B = 32
NCORES = 8
BPC = B // NCORES  # batches per core
SQRT_L = float(np.sqrt(float(L)))

CHUNK = 512  # projection / out matmul free dim
TCH = 128  # transpose chunk
TGRP = 8  # transposes per PSUM bank evacuation
IN_TILE = 4096  # HBM<->SBUF dma tile (2 MiB fp32)


def _patch_tail_drain(tile, mybir, ScopedClock):
    """This container's walrus rejects >1 sync wait on the kernel-tail Drain
    (setupSyncWait: 'Too many sync wait commands'). Spread the tail-drain
    waits across a chain of drains, one wait each."""
    if getattr(tile.TileContext, "_drain_split_patched", False):
        return

    def _split_drain_and_barrier(self, tick_clock, wait_clock):
        nc = self.nc
        drain_inst = nc.sync.drain()
        wait_clock.add_sem_waits(
            drain_inst.ins, ScopedClock({None: tick_clock.global_clock})
        )
        si = drain_inst.ins.sync_info
        if si is not None and si.on_wait is not None and len(si.on_wait) > 1:
            waits = list(si.on_wait)
            si.on_wait = waits[:1]
            for w in waits[1:]:
                extra = nc.sync.drain()
                esi = extra.ins.sync_info
                if esi is None:
                    extra.ins.sync_info = mybir.SyncInfo(on_wait=[w], on_update=[])
                else:
                    ow = list(esi.on_wait) if esi.on_wait else []
                    ow.append(w)
                    esi.on_wait = ow
        nc.all_engine_barrier()
        assert self.sems is not None
        popped = nc._tile_sem_poison_stack.pop()
        assert popped is self._sem_poison
        nc.clear_and_free_semaphores(list(self.sems.allocated().values()))
        nc.all_engine_barrier()

    tile.TileContext._drain_and_barrier = _split_drain_and_barrier
    tile.TileContext._drain_split_patched = True


def _split_excess_waits(nc, mybir, max_waits=1):
    """Walrus in this container rejects instructions carrying more than a
    couple of sync waits ('Too many sync wait commands'). Hoist excess waits
    onto dedicated same-engine NoOps inserted just before the instruction."""
    ctr = 0
    for blk in nc.m.functions[0].blocks:
        new_insts = []
        for inst in blk.instructions:
            si = inst.sync_info
            if si is not None and si.on_wait and len(si.on_wait) > max_waits:
                waits = list(si.on_wait)
                excess, keep = waits[:-max_waits], waits[-max_waits:]
                si.on_wait = keep
                for i in range(0, len(excess), max_waits):
                    ctr += 1
                    nop = mybir.InstNoOp(
                        name=f"I-waitsplit-{ctr}",
                        sync_info=mybir.SyncInfo(
                            on_wait=excess[i : i + max_waits], on_update=[]
                        ),
                        bass_nofuse=True,
                        engine=inst.engine,
                    )
                    nc.register_instruction(nop)
                    new_insts.append(nop)
            new_insts.append(inst)
        blk.instructions = new_insts


def build_nc(bpc=BPC, seq=L, scores_fp32=False):
    import concourse.bass as bass
    import concourse.mybir as mybir
    import concourse.tile as tile
    from concourse.masks import make_identity
    from concourse.vector_clock import ScopedClock

    _patch_tail_drain(tile, mybir, ScopedClock)

    f32 = mybir.dt.float32
    f32r = mybir.dt.float32r
    f16 = mybir.dt.float16
    AF = mybir.ActivationFunctionType
    ALU = mybir.AluOpType
    AX = mybir.AxisListType

    sqrt_l = float(np.sqrt(float(seq)))
    in_tile = min(IN_TILE, seq)
    ntr = seq // TCH  # number of 128-col transpose chunks
    tgrp = min(TGRP, ntr)  # transposes per PSUM bank
    nin = seq // in_tile  # dma tiles per batch
    cpin = in_tile // CHUNK  # matmul chunks per dma tile

    # dtype of the scores operand path (activations, transposes, scores mm)
    sdt = f32 if scores_fp32 else f16

    def r(ap):
        # reduced-precision single-pass view for fp32 matmul operands
        return ap.bitcast(f32r) if ap.dtype == f32 else ap

    nc = bass.Bass("TRN2", target_bir_lowering=False, debug=False)
    x_d = nc.dram_tensor("x", [bpc, P, seq], f32, kind="ExternalInput").ap()
    y_d = nc.dram_tensor("y", [bpc, P, seq], f32, kind="ExternalInput").ap()
    wf_d = nc.dram_tensor("wf", [P, P], f32, kind="ExternalInput").ap()
    bf_d = nc.dram_tensor("bf", [P], f32, kind="ExternalInput").ap()
    wa_d = nc.dram_tensor("wa", [P, P], f32, kind="ExternalInput").ap()
    ba_d = nc.dram_tensor("ba", [P], f32, kind="ExternalInput").ap()
    out_d = nc.dram_tensor("out", [bpc, P, seq], f32, kind="ExternalOutput").ap()

    with tile.TileContext(nc) as tc:
        with (
            tc.tile_pool(name="consts", bufs=1) as consts,
            tc.tile_pool(name="xin", bufs=3) as xin_pool,
            tc.tile_pool(name="acts", bufs=2) as acts_pool,
            tc.tile_pool(name="trs", bufs=1) as tr_pool,
            tc.tile_pool(name="sm", bufs=2) as sm_pool,
            tc.tile_pool(name="outs", bufs=2) as out_pool,
            tc.tile_pool(name="pproj", bufs=3, space="PSUM") as psum_proj,
            tc.tile_pool(name="ptr", bufs=2, space="PSUM") as psum_tr,
            tc.tile_pool(name="psc", bufs=1, space="PSUM") as psum_sc,
            tc.tile_pool(name="pout", bufs=2, space="PSUM") as psum_out,
        ):
            # Issue the first batch's input loads before anything else so the
            # DMA engines are saturated during constant setup (program order
            # drives scheduler priority).
            preloaded = {}
            for h in range(min(2, nin)):
                x_t = xin_pool.tile([P, in_tile], f32r, tag="x_t", name="x_t")
                y_t = xin_pool.tile([P, in_tile], f32r, tag="y_t", name="y_t")
                hs = slice(h * in_tile, (h + 1) * in_tile)
                nc.sync.dma_start(x_t, x_d[0, :, hs].bitcast(f32r))
                nc.sync.dma_start(y_t, y_d[0, :, hs].bitcast(f32r))
                preloaded[h] = (x_t, y_t)

            # ---- constants ----
            wf_nat = consts.tile([P, P], f32)
            nc.sync.dma_start(wf_nat, wf_d)
            wa_nat = consts.tile([P, P], f32)
            nc.sync.dma_start(wa_nat, wa_d)
            id32 = consts.tile([P, P], f32)
            make_identity(nc, id32)
            ids = consts.tile([P, P], sdt)
            make_identity(nc, ids)
            bias_f = consts.tile([P, 1], f32)
            nc.sync.dma_start(bias_f, bf_d.rearrange("(p o) -> p o", o=1))
            bias_a = consts.tile([P, 1], f32)
            nc.sync.dma_start(bias_a, ba_d.rearrange("(p o) -> p o", o=1))

            # WfT/WaT ([in,out] layout) via TensorE transpose; stored as
            # float32r so the BIR verifier accepts them as fp32r matmul
            # operands (producer must write the fp32r dtype).
            wfT = consts.tile([P, P], f32r)
            waT = consts.tile([P, P], f32r)
            for nat, tsp in ((wf_nat, wfT), (wa_nat, waT)):
                pw = psum_proj.tile([P, CHUNK], f32, tag="pp", name="pw")
                nc.tensor.transpose(pw[:, :P], nat, id32)
                nc.vector.tensor_copy(out=tsp, in_=pw[:, :P])

            for b in range(bpc):
                xp16 = acts_pool.tile([P, seq], sdt, tag="xp16", name="xp16")
                yp16 = acts_pool.tile([P, seq], sdt, tag="yp16", name="yp16")

                # ---- phase 1: stream in, project, bias + cast ----
                for h in range(nin):
                    if b == 0 and h in preloaded:
                        x_t, y_t = preloaded[h]
                    else:
                        # float32r tiles (same 4-byte layout as f32); the DRAM
                        # side is bitcast so the DMA is dtype-consistent and
                        # the fp32r matmul sees a properly-typed producer.
                        x_t = xin_pool.tile([P, in_tile], f32r, tag="x_t", name="x_t")
                        y_t = xin_pool.tile([P, in_tile], f32r, tag="y_t", name="y_t")
                        hs = slice(h * in_tile, (h + 1) * in_tile)
                        nc.sync.dma_start(x_t, x_d[b, :, hs].bitcast(f32r))
                        nc.sync.dma_start(y_t, y_d[b, :, hs].bitcast(f32r))
                    for cc in range(cpin):
                        c0 = h * in_tile + cc * CHUNK
                        cs_in = slice(cc * CHUNK, (cc + 1) * CHUNK)
                        cs = slice(c0, c0 + CHUNK)
                        px = psum_proj.tile([P, CHUNK], f32, tag="pp", name="px")
                        nc.tensor.matmul(
                            px, r(wfT[:]), r(x_t[:, cs_in]), start=True, stop=True
                        )
                        nc.scalar.activation(
                            out=xp16[:, cs], in_=px, func=AF.Identity, bias=bias_f
                        )
                        py = psum_proj.tile([P, CHUNK], f32, tag="pp", name="py")
                        nc.tensor.matmul(
                            py, r(waT[:]), r(y_t[:, cs_in]), start=True, stop=True
                        )
                        nc.scalar.activation(
                            out=yp16[:, cs], in_=py, func=AF.Identity, bias=bias_a
                        )

                # ---- phase 2: transpose activations ----
                xpT = tr_pool.tile([P, seq], sdt, tag="xpT", name="xpT")
                ypT = tr_pool.tile([P, seq], sdt, tag="ypT", name="ypT")
                for g in range(ntr // tgrp):
                    ptx = psum_tr.tile([P, tgrp * TCH], sdt, tag="pt", name="ptx")
                    pty = psum_tr.tile([P, tgrp * TCH], sdt, tag="pt", name="pty")
                    for t in range(tgrp):
                        c = g * tgrp + t
                        ts_ = slice(t * TCH, (t + 1) * TCH)
                        cs = slice(c * TCH, (c + 1) * TCH)
                        nc.tensor.transpose(ptx[:, ts_], xp16[:, cs], ids)
                        nc.tensor.transpose(pty[:, ts_], yp16[:, cs], ids)
                    gs = slice(g * tgrp * TCH, (g + 1) * tgrp * TCH)
                    nc.vector.tensor_copy(out=xpT[:, gs], in_=ptx)
                    nc.vector.tensor_copy(out=ypT[:, gs], in_=pty)

                # ---- phase 3: scores (accumulate over seq chunks) ----
                ps = psum_sc.tile([P, P], f32, tag="ps", name="ps")
                for c in range(ntr):
                    cs = slice(c * TCH, (c + 1) * TCH)
                    nc.tensor.matmul(
                        ps,
                        xpT[:, cs],
                        ypT[:, cs],
                        start=(c == 0),
                        stop=(c == ntr - 1),
                    )

                # ---- phase 4: softmax (rowwise over free dim) ----
                negmx = sm_pool.tile([P, 1], f32, tag="negmx", name="negmx")
                nc.vector.tensor_reduce(
                    out=negmx, in_=ps, axis=AX.X, op=ALU.max, negate=True
                )
                e = sm_pool.tile([P, P], f32, tag="e", name="e")
                se = sm_pool.tile([P, 1], f32, tag="se", name="se")
                nc.scalar.activation(
                    out=e, in_=ps, func=AF.Exp, bias=negmx, scale=1.0, accum_out=se
                )
                sse = sm_pool.tile([P, 1], f32, tag="sse", name="sse")
                nc.vector.tensor_scalar_mul(sse, se, sqrt_l)
                rcp = sm_pool.tile([P, 1], f32, tag="rcp", name="rcp")
                nc.vector.reciprocal(rcp, sse)
                attn = sm_pool.tile([P, P], sdt, tag="attn", name="attn")
                nc.vector.tensor_scalar_mul(attn, e, rcp)
                pat = psum_tr.tile([P, tgrp * TCH], sdt, tag="pt", name="pat")
                nc.tensor.transpose(pat[:, :P], attn, ids)
                attnT = sm_pool.tile([P, P], sdt, tag="attnT", name="attnT")
                nc.vector.tensor_copy(out=attnT, in_=pat[:, :P])

                # ---- phase 5: out = attnT.T @ (xp + yp), stream back ----
                out_tile = min(1024, seq)
                nout = seq // out_tile
                cpo = out_tile // CHUNK
                for h in range(nout):
                    ot = out_pool.tile([P, out_tile], f32, tag="ot", name="ot")
                    for cc in range(cpo):
                        c0 = h * out_tile + cc * CHUNK
                        cs = slice(c0, c0 + CHUNK)
                        po = psum_out.tile([P, CHUNK], f32, tag="po", name="po")
                        nc.tensor.matmul(
                            po, attnT[:], xp16[:, cs], start=True, stop=False
                        )
                        nc.tensor.matmul(
                            po, attnT[:], yp16[:, cs], start=False, stop=True
                        )
                        nc.any.tensor_copy(
                            out=ot[:, cc * CHUNK : (cc + 1) * CHUNK], in_=po
                        )
                    hs = slice(h * out_tile, (h + 1) * out_tile)
                    # stores issue from the ACT HWDGE ring so they don't
                    # share the SP ring with input loads
                    nc.scalar.dma_start(out_d[b, :, hs], ot)

    _split_excess_waits(nc, mybir, max_waits=1)
    return nc


_nc_cache = {}


def _get_nc():
    key = (BPC, L)
    if key not in _nc_cache:
        _nc_cache[key] = build_nc(BPC, L)
    return _nc_cache[key]


def kernel(x, y, Wf, bf, Wa, ba):
    from concourse.bass_utils import run_bass_kernel_spmd

    x = np.asarray(x, dtype=np.float32)
    y = np.asarray(y, dtype=np.float32)
    Wf = np.ascontiguousarray(np.asarray(Wf, dtype=np.float32))
    bf = np.ascontiguousarray(np.asarray(bf, dtype=np.float32))
    Wa = np.ascontiguousarray(np.asarray(Wa, dtype=np.float32))
    ba = np.ascontiguousarray(np.asarray(ba, dtype=np.float32))

    nc = _get_nc()
    in_maps = []
    for c in range(NCORES):
        sl = slice(c * BPC, (c + 1) * BPC)
        in_maps.append(
            {
                "x": np.ascontiguousarray(x[sl]),
                "y": np.ascontiguousarray(y[sl]),
                "wf": Wf,
                "bf": bf,
                "wa": Wa,
                "ba": ba,
            }
        )
    res = run_bass_kernel_spmd(nc, in_maps, core_ids=list(range(NCORES)))
    out = np.concatenate([r["out"] for r in res.results], axis=0)
    return np.ascontiguousarray(out.astype(np.float32))


if __name__ == "__main__":
    rng = np.random.default_rng(0)
    inputs = {
        "x": rng.standard_normal((B, P, L), dtype=np.float32),
        "y": rng.standard_normal((B, P, L), dtype=np.float32),
        "Wf": (rng.standard_normal((P, P)) / np.sqrt(P)).astype(np.float32),
        "bf": (rng.standard_normal(P) * 0.02).astype(np.float32),
        "Wa": (rng.standard_normal((P, P)) / np.sqrt(P)).astype(np.float32),
        "ba": (rng.standard_normal(P) * 0.02).astype(np.float32),
    }
    o = kernel(**inputs)
    print(o.shape, o.dtype)



# revision 12
# speedup vs baseline: 1.5519x; 1.5519x over previous
"""Biaffine attention kernel for Trainium2, data-parallel over 8 NeuronCores.

Math (per batch b):
    xp = Wf @ x[b] + bf          (128, L)
    yp = Wa @ y[b] + ba          (128, L)
    scores = xp @ yp.T           (128, 128)   contraction over L
    attn = softmax(scores, -1) / sqrt(L)
    out[b] = attn @ (xp + yp)    (128, L)

Distribution: batch dim (32) sharded 4-per-core across 8 cores; weights
replicated. No collectives.

Key structure (all fp16 on the wire, fp32 PSUM accumulation):
  - x/y staged to HBM as fp16 (host cast): halves input DMA traffic.
  - Transposed activations xpT/ypT computed DIRECTLY on TensorE with the
    input chunk as the stationary operand (out = x_c.T @ WfT), skipping
    the separate natural-projection + transpose passes entirely.
  - Bias added during PSUM evacuation as a broadcast row tensor
    (scalar_tensor_tensor on DVE / GpSimd), so scores need no fixups.
  - scores accumulate over 64 chunk matmuls into one PSUM bank.
  - Softmax rowwise: DVE max-reduce, ACT exp with fused -max bias and
    sum accumulation, DVE reciprocal; 1/sqrt(L) folded in.
  - Output fused: out = attn@(xp+yp) = (attn@Wf)@x + (attn@Wa)@y
    + (attn@(bf+ba)) broadcast. AfT/AaT are tiny 128x128 matmuls from
    attnT; z is never materialized; the bias term rides the ACT
    evacuation as a per-partition bias. Output stored fp16, host upcast.
"""

import numpy as np

P = 128
L = 8192
B = 32
NCORES = 8
BPC = B // NCORES  # batches per core
SQRT_L = float(np.sqrt(float(L)))

IN_TILE = 2048  # HBM->SBUF dma tile (4 KiB/partition fp16)
GRP = 512  # pass-B PSUM group (4 x 128-col chunks per bank)
OUT_TILE = 2048  # SBUF->HBM out staging tile


def _patch_tail_drain(tile, mybir, ScopedClock):
    """This container's walrus rejects >1 sync wait on the kernel-tail Drain
    (setupSyncWait: 'Too many sync wait commands'). Spread the tail-drain
    waits across a chain of drains, one wait each."""
    if getattr(tile.TileContext, "_drain_split_patched", False):
        return

    def _split_drain_and_barrier(self, tick_clock, wait_clock):
        nc = self.nc
        drain_inst = nc.sync.drain()
        wait_clock.add_sem_waits(
            drain_inst.ins, ScopedClock({None: tick_clock.global_clock})
        )
        si = drain_inst.ins.sync_info
        if si is not None and si.on_wait is not None and len(si.on_wait) > 1:
            waits = list(si.on_wait)
            si.on_wait = waits[:1]
            for w in waits[1:]:
                extra = nc.sync.drain()
                esi = extra.ins.sync_info
                if esi is None:
                    extra.ins.sync_info = mybir.SyncInfo(on_wait=[w], on_update=[])
                else:
                    ow = list(esi.on_wait) if esi.on_wait else []
                    ow.append(w)
                    esi.on_wait = ow
        nc.all_engine_barrier()
        assert self.sems is not None
        popped = nc._tile_sem_poison_stack.pop()
        assert popped is self._sem_poison
        nc.clear_and_free_semaphores(list(self.sems.allocated().values()))
        nc.all_engine_barrier()

    tile.TileContext._drain_and_barrier = _split_drain_and_barrier
    tile.TileContext._drain_split_patched = True


def _split_excess_waits(nc, mybir, max_waits=1):
    """Walrus in this container rejects instructions carrying more than a
    couple of sync waits ('Too many sync wait commands'). Hoist excess waits
    onto dedicated same-engine NoOps inserted just before the instruction."""
    ctr = 0
    for blk in nc.m.functions[0].blocks:
        new_insts = []
        for inst in blk.instructions:
            si = inst.sync_info
            if si is not None and si.on_wait and len(si.on_wait) > max_waits:
                waits = list(si.on_wait)
                excess, keep = waits[:-max_waits], waits[-max_waits:]
                si.on_wait = keep
                for i in range(0, len(excess), max_waits):
                    ctr += 1
                    nop = mybir.InstNoOp(
                        name=f"I-waitsplit-{ctr}",
                        sync_info=mybir.SyncInfo(
                            on_wait=excess[i : i + max_waits], on_update=[]
                        ),
                        bass_nofuse=True,
                        engine=inst.engine,
                    )
                    nc.register_instruction(nop)
                    new_insts.append(nop)
            new_insts.append(inst)
        blk.instructions = new_insts


def build_nc(bpc=BPC, seq=L):
    import concourse.bass as bass
    import concourse.mybir as mybir
    import concourse.tile as tile
    from concourse.masks import make_identity
    from concourse.vector_clock import ScopedClock

    _patch_tail_drain(tile, mybir, ScopedClock)

    f32 = mybir.dt.float32
    f16 = mybir.dt.float16
    AF = mybir.ActivationFunctionType
    ALU = mybir.AluOpType
    AX = mybir.AxisListType

    sqrt_l = float(np.sqrt(float(seq)))
    nin = seq // IN_TILE  # dma tiles per batch tensor
    ngrp = seq // GRP  # pass-B psum groups per batch
    gpc = IN_TILE // GRP  # groups per dma tile
    ntr = seq // P  # 128-col chunks per batch

    nc = bass.Bass("TRN2", target_bir_lowering=False, debug=False)
    x_d = nc.dram_tensor("x", [bpc, P, seq], f16, kind="ExternalInput").ap()
    y_d = nc.dram_tensor("y", [bpc, P, seq], f16, kind="ExternalInput").ap()
    wf_d = nc.dram_tensor("wf", [P, P], f16, kind="ExternalInput").ap()
    wa_d = nc.dram_tensor("wa", [P, P], f16, kind="ExternalInput").ap()
    wft_d = nc.dram_tensor("wft", [P, P], f16, kind="ExternalInput").ap()
    wat_d = nc.dram_tensor("wat", [P, P], f16, kind="ExternalInput").ap()
    bfb_d = nc.dram_tensor("bfb", [P, GRP], f16, kind="ExternalInput").ap()
    bab_d = nc.dram_tensor("bab", [P, GRP], f16, kind="ExternalInput").ap()
    bzb_d = nc.dram_tensor("bzb", [P, P], f16, kind="ExternalInput").ap()
    out_d = nc.dram_tensor("out", [bpc, P, seq], f16, kind="ExternalOutput").ap()

    with tile.TileContext(nc) as tc:
        with (
            tc.tile_pool(name="consts", bufs=1) as consts,
            tc.tile_pool(name="xin", bufs=2) as xin_pool,
            tc.tile_pool(name="acts", bufs=2) as acts_pool,
            tc.tile_pool(name="sm", bufs=2) as sm_pool,
            tc.tile_pool(name="outs", bufs=2) as out_pool,
            tc.tile_pool(name="pxt", bufs=2, space="PSUM") as psum_xt,
            tc.tile_pool(name="pyt", bufs=2, space="PSUM") as psum_yt,
            tc.tile_pool(name="psc", bufs=1, space="PSUM") as psum_sc,
            tc.tile_pool(name="pout", bufs=2, space="PSUM") as psum_out,
            tc.tile_pool(name="psm", bufs=1, space="PSUM") as psum_sm,
        ):
            # Issue the first batch's input loads before anything else so
            # the DMA engines are saturated during constant setup (program
            # order drives scheduler priority). x on the SP ring, y on the
            # GpSimd ring so the two streams ride separate queues.
            preloaded = {}
            for h in range(nin):
                x_t = xin_pool.tile([P, IN_TILE], f16, tag=f"x{h}", name=f"x{h}")
                y_t = xin_pool.tile([P, IN_TILE], f16, tag=f"y{h}", name=f"y{h}")
                hs = slice(h * IN_TILE, (h + 1) * IN_TILE)
                nc.sync.dma_start(x_t, x_d[0, :, hs])
                nc.gpsimd.dma_start(y_t, y_d[0, :, hs])
                preloaded[h] = (x_t, y_t)

            # ---- constants ----
            wf16 = consts.tile([P, P], f16)
            nc.sync.dma_start(wf16, wf_d)
            wa16 = consts.tile([P, P], f16)
            nc.sync.dma_start(wa16, wa_d)
            wfT = consts.tile([P, P], f16)
            nc.sync.dma_start(wfT, wft_d)
            waT = consts.tile([P, P], f16)
            nc.sync.dma_start(waT, wat_d)
            bfb = consts.tile([P, GRP], f16)
            nc.sync.dma_start(bfb, bfb_d)
            bab = consts.tile([P, GRP], f16)
            nc.sync.dma_start(bab, bab_d)
            bzb = consts.tile([P, P], f16)
            nc.sync.dma_start(bzb, bzb_d)
            ids = consts.tile([P, P], f16)
            make_identity(nc, ids)

            for b in range(bpc):
                # ---- phase 1: stream inputs ----
                if b == 0:
                    xt = [preloaded[h][0] for h in range(nin)]
                    yt = [preloaded[h][1] for h in range(nin)]
                else:
                    xt, yt = [], []
                    for h in range(nin):
                        x_t = xin_pool.tile(
                            [P, IN_TILE], f16, tag=f"x{h}", name=f"x{h}"
                        )
                        y_t = xin_pool.tile(
                            [P, IN_TILE], f16, tag=f"y{h}", name=f"y{h}"
                        )
                        hs = slice(h * IN_TILE, (h + 1) * IN_TILE)
                        nc.sync.dma_start(x_t, x_d[b, :, hs])
                        nc.gpsimd.dma_start(y_t, y_d[b, :, hs])
                        xt.append(x_t)
                        yt.append(y_t)

                # ---- phase 2: transposed projections (direct) ----
                # xpT[:, c*128:(c+1)*128] = x_c.T @ WfT  (+ bf broadcast)
                xpT = acts_pool.tile([P, seq], f16, tag="xpT", name="xpT")
                ypT = acts_pool.tile([P, seq], f16, tag="ypT", name="ypT")
                for g in range(ngrp):
                    ht = g // gpc  # which dma tile
                    px = psum_xt.tile([P, GRP], f32, tag="px", name="px")
                    py = psum_yt.tile([P, GRP], f32, tag="py", name="py")
                    for t in range(4):
                        lo = (g % gpc) * GRP + t * P
                        cs = slice(lo, lo + P)
                        ts_ = slice(t * P, (t + 1) * P)
                        nc.tensor.matmul(
                            px[:, ts_], xt[ht][:, cs], wfT, start=True, stop=True
                        )
                    for t in range(4):
                        lo = (g % gpc) * GRP + t * P
                        cs = slice(lo, lo + P)
                        ts_ = slice(t * P, (t + 1) * P)
                        nc.tensor.matmul(
                            py[:, ts_], yt[ht][:, cs], waT, start=True, stop=True
                        )
                    gs = slice(g * GRP, (g + 1) * GRP)
                    nc.vector.scalar_tensor_tensor(
                        out=xpT[:, gs],
                        in0=px,
                        scalar=1.0,
                        in1=bfb,
                        op0=ALU.mult,
                        op1=ALU.add,
                    )
                    nc.vector.scalar_tensor_tensor(
                        out=ypT[:, gs],
                        in0=py,
                        scalar=1.0,
                        in1=bab,
                        op0=ALU.mult,
                        op1=ALU.add,
                    )

                # ---- phase 3: scores (accumulate over seq chunks) ----
                ps = psum_sc.tile([P, P], f32, tag="ps", name="ps")
                for c in range(ntr):
                    cs = slice(c * P, (c + 1) * P)
                    nc.tensor.matmul(
                        ps,
                        xpT[:, cs],
                        ypT[:, cs],
                        start=(c == 0),
                        stop=(c == ntr - 1),
                    )

                # ---- phase 4: softmax (rowwise over free dim) ----
                negmx = sm_pool.tile([P, 1], f32, tag="negmx", name="negmx")
                nc.vector.tensor_reduce(
                    out=negmx, in_=ps, axis=AX.X, op=ALU.max, negate=True
                )
                e = sm_pool.tile([P, P], f32, tag="e", name="e")
                se = sm_pool.tile([P, 1], f32, tag="se", name="se")
                nc.scalar.activation(
                    out=e, in_=ps, func=AF.Exp, bias=negmx, scale=1.0, accum_out=se
                )
                sse = sm_pool.tile([P, 1], f32, tag="sse", name="sse")
                nc.vector.tensor_scalar_mul(sse, se, sqrt_l)
                rcp = sm_pool.tile([P, 1], f32, tag="rcp", name="rcp")
                nc.vector.reciprocal(rcp, sse)
                attn = sm_pool.tile([P, P], f16, tag="attn", name="attn")
                nc.vector.tensor_scalar_mul(attn, e, rcp)
                # ab = attn @ (bf+ba): free-dim weighted row-sum on DVE
                # (bzb broadcasts bz along partitions); junk is scratch.
                junk = sm_pool.tile([P, P], f16, tag="junk", name="junk")
                ab = sm_pool.tile([P, 1], f32, tag="ab", name="ab")
                nc.vector.scalar_tensor_tensor(
                    out=junk,
                    in0=attn,
                    scalar=1.0,
                    in1=bzb,
                    op0=ALU.mult,
                    op1=ALU.mult,
                    accum_out=ab,
                )
                pat = psum_sm.tile([P, P], f16, tag="pat", name="pat")
                nc.tensor.transpose(pat, attn, ids)
                attnT = sm_pool.tile([P, P], f16, tag="attnT", name="attnT")
                nc.vector.tensor_copy(out=attnT, in_=pat)

                # ---- phase 5: fused-out prep ----
                # AfT = (attn @ Wf).T = Wf(lhsT).T @ attnT ; same for AaT.
                # paf/paa ride segments of a pout-pool tile (no extra bank).
                pwt = psum_out.tile([P, GRP], f32, tag="po", name="pwt")
                nc.tensor.matmul(pwt[:, 0:P], wf16, attnT, start=True, stop=True)
                AfT = sm_pool.tile([P, P], f16, tag="AfT", name="AfT")
                nc.vector.tensor_copy(out=AfT, in_=pwt[:, 0:P])
                nc.tensor.matmul(
                    pwt[:, P : 2 * P], wa16, attnT, start=True, stop=True
                )
                AaT = sm_pool.tile([P, P], f16, tag="AaT", name="AaT")
                nc.vector.tensor_copy(out=AaT, in_=pwt[:, P : 2 * P])

                # ---- phase 6: out = AfT.T @ x + AaT.T @ y (+ab), stream ----
                nout = seq // OUT_TILE
                cpo = OUT_TILE // GRP
                for h in range(nout):
                    ot = out_pool.tile([P, OUT_TILE], f16, tag="ot", name="ot")
                    for cc in range(cpo):
                        c0 = h * OUT_TILE + cc * GRP
                        ht = c0 // IN_TILE
                        lo = c0 % IN_TILE
                        cs = slice(lo, lo + GRP)
                        po = psum_out.tile([P, GRP], f32, tag="po", name="po")
                        nc.tensor.matmul(
                            po, AfT, xt[ht][:, cs], start=True, stop=False
                        )
                        nc.tensor.matmul(
                            po, AaT, yt[ht][:, cs], start=False, stop=True
                        )
                        nc.scalar.activation(
                            out=ot[:, cc * GRP : (cc + 1) * GRP],
                            in_=po,
                            func=AF.Identity,
                            bias=ab,
                        )
                    hs = slice(h * OUT_TILE, (h + 1) * OUT_TILE)
                    # stores issue from the ACT HWDGE ring so they don't
                    # share the SP ring with input loads
                    nc.scalar.dma_start(out_d[b, :, hs], ot)

    _split_excess_waits(nc, mybir, max_waits=1)
    return nc


_nc_cache = {}


def _get_nc():
    key = (BPC, L)
    if key not in _nc_cache:
        _nc_cache[key] = build_nc(BPC, L)
    return _nc_cache[key]


def make_in_maps(x, y, Wf, bf, Wa, ba):
    """Host staging: fp16 casts + layout-only prep, sharded per core."""
    x16 = np.asarray(x, dtype=np.float16)
    y16 = np.asarray(y, dtype=np.float16)
    Wf = np.asarray(Wf, dtype=np.float32)
    bf = np.asarray(bf, dtype=np.float32)
    Wa = np.asarray(Wa, dtype=np.float32)
    ba = np.asarray(ba, dtype=np.float32)

    wf16 = np.ascontiguousarray(Wf.astype(np.float16))
    wa16 = np.ascontiguousarray(Wa.astype(np.float16))
    wft16 = np.ascontiguousarray(Wf.T.astype(np.float16))
    wat16 = np.ascontiguousarray(Wa.T.astype(np.float16))
    bfb = np.ascontiguousarray(np.tile(bf.astype(np.float16), (P, GRP // P)))
    bab = np.ascontiguousarray(np.tile(ba.astype(np.float16), (P, GRP // P)))
    bzb = np.ascontiguousarray(np.tile((bf + ba).astype(np.float16), (P, 1)))

    in_maps = []
    for c in range(NCORES):
        sl = slice(c * BPC, (c + 1) * BPC)
        in_maps.append(
            {
                "x": np.ascontiguousarray(x16[sl]),
                "y": np.ascontiguousarray(y16[sl]),
                "wf": wf16,
                "wa": wa16,
                "wft": wft16,
                "wat": wat16,
                "bfb": bfb,
                "bab": bab,
                "bzb": bzb,
            }
        )
    return in_maps


def kernel(x, y, Wf, bf, Wa, ba):
    from concourse.bass_utils import run_bass_kernel_spmd

    nc = _get_nc()
    in_maps = make_in_maps(x, y, Wf, bf, Wa, ba)
    res = run_bass_kernel_spmd(nc, in_maps, core_ids=list(range(NCORES)))
    out = np.concatenate([r["out"] for r in res.results], axis=0)
    return np.ascontiguousarray(out.astype(np.float32))


if __name__ == "__main__":
    rng = np.random.default_rng(0)
    inputs = {
        "x": rng.standard_normal((B, P, L), dtype=np.float32),
        "y": rng.standard_normal((B, P, L), dtype=np.float32),
        "Wf": (rng.standard_normal((P, P)) / np.sqrt(P)).astype(np.float32),
        "bf": (rng.standard_normal(P) * 0.02).astype(np.float32),
        "Wa": (rng.standard_normal((P, P)) / np.sqrt(P)).astype(np.float32),
        "ba": (rng.standard_normal(P) * 0.02).astype(np.float32),
    }
    o = kernel(**inputs)
    print(o.shape, o.dtype)
